# revision 32
# baseline (speedup 1.0000x reference)
"""Conv2d 3x3 (stride 1, pad 1) as implicit GEMM on 8 Trainium2 NeuronCores.

Problem: x [32,128,56,56] f32, weight [256,128,3,3] f32, bias [256] f32
         -> out [32,256,56,56] f32.

Sharding: data-parallel over batch. Each of the 8 cores gets 4 images;
weight/bias are replicated. No collectives; outputs are concatenated on host.

Per-core kernel (implicit GEMM, bf16 operands, fp32 PSUM accumulate):
  - x is host-padded to bf16 [4,128,58,64]: 1px conv halo + row pitch padded
    56->64 so each row slice is 128B (16B-line aligned) in SBUF.
  - weight is host-rearranged to bf16 [128, 9, 256] (in-ch partitions, 3x3
    taps, out-ch free) so lhsT slices need no on-device transpose.
  - For each image, out-channel group g (2 groups of 128) and band of 8
    output rows (7 bands): accumulate 9 matmuls (one per tap) into a
    [128, 448] PSUM tile: psum += W[:, ki, g*128:...].T @ xpad[:, rows+kh, kw:kw+56]
  - bias-add + PSUM->SBUF(bf16) on scalar/vector engines alternating,
    then DMA to DRAM (bf16). Host converts the gathered output to f32.

Measured cadence ladder at N=448 (trn2 NTFF/perfetto):
  - fp32r: 210ns/MM - LDWEIGHTS-bound (fp32 LDW 187ns + ~20ns handover
    exceeds the 186.7ns stream; fp32r cannot use standalone ldweights).
  - bf16 unpadded rows: 230ns/MM (LDW 116ns hidden; +43ns suspected from
    the 8-row rhs AP blocks at 116B stride crossing 16B SBUF lines).
  - bf16 with 128B-aligned rows: this kernel. Target ~190ns/MM.
  Per-matmul tick-sem increments are stripped down to group-final matmuls
  (26.3ns/inc measured, though removing them alone did not change cadence).

Other head/tail measures:
  - Warm-up matmuls run on a memset tile (no DMA dependency) so the PE's
    HAM clock-gate (1.2 -> 2.4 GHz, fires ~2 activity windows after PE
    work starts) warms while the first input chunk is in flight.
  - DMA triggers cost ~650ns each on their (sync/scalar) queue and Tile
    rotates ~10 completion-sem slots, so the head issues exactly 10 DMAs:
    7 image-0 chunks on sync, w-group0 / bias / w-group1 bundles on
    scalar. Later images trickle in coarser chunks.
"""

import numpy as np
import ml_dtypes

import concourse.bacc as bacc
import concourse.mybir as mybir
import concourse.tile as tile
from concourse.bass_utils import run_bass_kernel_spmd

N_CORES = 8
B, C_IN, H, W = 32, 128, 56, 56
C_OUT = 256
KH = KW = 3
B_LOC = B // N_CORES          # 4 images per core
HP = H + 2                    # 58 padded rows (conv halo)
WP = W + 2                    # 58 valid padded cols
WPAD = 64                     # row pitch in SBUF: 64 bf16 = 128B, 16B-aligned
ROWS = 8                      # output rows per matmul
NCHUNK = H // ROWS            # 7 bands
NFREE = ROWS * W              # 448 = matmul free dim (fits one PSUM bank)
NGRP = C_OUT // 128           # 2 out-channel groups

MM_DT = mybir.dt.bfloat16
OUT_DT = mybir.dt.bfloat16
BF16 = ml_dtypes.bfloat16
F8 = ml_dtypes.float8_e4m3
F8_DT = mybir.dt.float8e4
X8P = 2 * W                   # fp8 interleaved row pitch: 112 bytes
# The two center-column taps (kh=0,kw=1),(kh=1,kw=1) run as ONE fp8
# DoubleRow matmul (contraction 256/cycle). The rhs pair elements MUST be
# physically adjacent bytes (a stride-56 pair AP streamed at byte rate,
# ~360ns/MM; adjacent pairs stream 2B/cycle like bf16): host builds an
# interleaved copy x8i[r, c, i] = x8pad[r+i, c] (pitch 112, vertical halo
# baked in), rhs AP [128, (2 @1), (448 @2)]; weights pair [cin, 2, cout].
# fp8 e4m3 cannot represent the raw weights (std 0.01 is subnormal), so
# host scales w*8 and x/8 - the product scale cancels exactly. Error
# budget: 2/9 of the contraction in fp8 adds ~1.4e-2 L2 rel err (gate
# 2e-2; bf16 alone is 2.7e-3).
BF16_TAPS = [(0, 0), (0, 2), (1, 0), (1, 2), (2, 0), (2, 1), (2, 2)]

N_WARM = 68                   # warm-up matmuls (N=64); last 23 are DMA hooks.
                              # Calibrated so the chain ends AT data-ready
                              # (~10.2us): hooks then release the prefetch
                              # flood only after the critical head transfers'
                              # sems have fired (zero pollution window), and
                              # the HAM clock-gate (3.4us sustained activity)
                              # fires before the real stream starts.
NWU = 64                      # warm-up matmul free dim (small => fine-grained,
                              # ~57ns each cold; bridges preamble-end to
                              # first-data with no >µs PE idle hole, so the
                              # HAM activity window is never reset)


def _strip_mm_tick_updates(nc):
    """Remove the Tile tick-sem increment from non-stop matmuls.

    Tile attaches a `sem-inc @complete` to EVERY matmul so consumers can
    wait "first v matmuls done". Matmuls complete in queue order, so it is
    sufficient for only accumulation-group-final (stop=True) matmuls to
    increment, with every wait value remapped from "v matmuls" to "k
    stop-matmuls". All Tile-emitted waits on the tick sem land on group
    boundaries (asserted below), because the only cross-engine consumers
    of matmul completion are whole-PSUM-tile readers.
    """
    import concourse.mybir as mybir
    from collections import Counter

    mms = []
    others = []
    for f in nc.m.functions:
        for blk in f.blocks:
            for inst in blk.instructions:
                if isinstance(inst, mybir.InstMatmult):
                    mms.append(inst)
                else:
                    others.append(inst)
    ids = Counter()
    for m in mms:
        si = m.sync_info
        for u in si.on_update if si else []:
            if u.update_mode == "sem-inc":
                ids[u.id] += 1
    if not ids:
        return
    tick, cnt = ids.most_common(1)[0]
    assert cnt == len(mms), (tick, cnt, len(mms))

    kept = [bool(m.stop_tensor_calc) for m in mms]
    prefix = [0]
    for k in kept:
        prefix.append(prefix[-1] + (1 if k else 0))

    def remap(v):
        assert 0 <= v <= len(mms), v
        # wait must land on a stop-matmul boundary, else ordering is lost
        assert v == 0 or kept[v - 1], f"tick wait {v} not at a group boundary"
        return prefix[v]

    for inst in mms + others:
        si = getattr(inst, "sync_info", None)
        if not si:
            continue
        for w in si.on_wait:
            if w.sync_type == "semaphore" and w.id == tick:
                assert w.wait_mode == "sem-ge-imm", w
                w.wait_value = remap(w.wait_value)
    for m, k in zip(mms, kept):
        if not k:
            si = m.sync_info
            si.on_update = [
                u
                for u in si.on_update
                if not (u.sync_type == "semaphore" and u.id == tick)
            ]


def _build():
    nc = bacc.Bacc(None, target_bir_lowering=False)
    xp = nc.dram_tensor("xp", [B_LOC, C_IN, HP, WPAD], MM_DT, kind="ExternalInput")
    # weights laid out [cin, group, tap, 128] so any (group, tap-range)
    # slice is CONTIGUOUS per partition: the previous [cin, tap, cout]
    # layout made head weight-DMAs 256B-packet sprays (measured ~4us to
    # land); contiguous slices move as 768-2304B packets.
    wt = nc.dram_tensor(
        "wt", [C_IN, NGRP, KH * KW, 128], MM_DT, kind="ExternalInput"
    )
    xp8 = nc.dram_tensor("xp8", [B_LOC, C_IN, H, X8P], F8_DT, kind="ExternalInput")
    w8 = nc.dram_tensor("w8", [C_IN, NGRP, 2, 128], F8_DT, kind="ExternalInput")
    bz = nc.dram_tensor("bz", [128, NGRP], mybir.dt.float32, kind="ExternalInput")
    out = nc.dram_tensor(
        "out", [B_LOC, NGRP, 128, H * W], OUT_DT, kind="ExternalOutput"
    )

    with tile.TileContext(nc) as tc:
        with (
            tc.tile_pool(name="const", bufs=1) as cpool,
            tc.tile_pool(name="xin", bufs=B_LOC) as xpool,
            tc.tile_pool(name="oout", bufs=8) as opool,
            tc.tile_pool(name="psum", bufs=1, space="PSUM") as pspool,
        ):
            w_tile = cpool.tile([C_IN, NGRP, KH * KW, 128], MM_DT)
            w8_tile = cpool.tile([C_IN, NGRP, 2, 128], F8_DT)
            b_tile = cpool.tile([128, NGRP], mybir.dt.float32)
            x_tiles = [
                xpool.tile([C_IN, HP, WPAD], MM_DT, name=f"x_img{b}", tag="ximg")
                for b in range(B_LOC)
            ]
            x8_tiles = [
                xpool.tile([C_IN, H, X8P], F8_DT, name=f"x8_img{b}", tag="x8img")
                for b in range(B_LOC)
            ]

            # PE warm-up: many small (N=64) matmuls so the HAM clock-gate
            # sees continuous PE activity from preamble-end until the first
            # input chunk has landed (the 4096-cycle HAM window is reset by
            # ~1us idle holes, so coverage must be gapless; small matmuls
            # drain ~55ns each once real data is ready, so they barely
            # delay the real stream). The LAST warmups double as DMA
            # release hooks: each reads one 64-element row of a prefetch
            # DMA's destination region (uninitialized SBUF - the product is
            # garbage into a never-read PSUM bank, which is fine). The WAR
            # dependency forces that DMA trigger to wait for the hook
            # matmul, so prefetch transfers cannot hoist to the head and
            # round-robin-steal queue bandwidth from the critical
            # chunk0/weight transfers (a DMA's completion sem only fires
            # when the WHOLE transfer has landed, so co-pending bytes
            # directly delay the first real matmul).
            wu = cpool.tile([128, NWU], MM_DT)
            wu8 = cpool.tile([128, NWU], F8_DT)
            nc.gpsimd.memset(wu[:], 0.0)
            nc.gpsimd.memset(wu8[:], 0.0)
            wu_ps = pspool.tile([64, NWU], mybir.dt.float32, tag="warm", bufs=1)
            # hook list, in release order (earliest-needed DMA first):
            # (kind, img, row) read by the hook warmup; the DMA writing
            # that row region is issued below and inherits the WAR dep.
            hooks = [
                ("x", 0, 10),   # chunk(0,1)  rows 10:18
                ("x", 0, 18),   # chunk(0,2)  rows 18:26
                ("x", 0, 26),   # chunk(0,3)  rows 26:34
                ("x", 1, 0),    # img1 chunk0 rows  0:18
                ("x", 0, 34),   # chunk(0,4)  rows 34:42
                ("f", 0, 0),    # fp8 img0 rows  0:8
                ("w8", 0, 0),   # fp8 DR weights (both groups)
                ("x", 0, 42),   # chunk(0,5)  rows 42:50
                ("f", 0, 8),    # fp8 img0 rows 8:24
                ("x", 0, 50),   # chunk(0,6)  rows 50:58
                ("x", 1, 18),   # img1 chunk1 rows 18:38
                ("w", 1, 0),    # w g1 (all taps)
                ("f", 0, 24),   # fp8 img0 rows 24:56
                ("x", 1, 38),   # img1 chunk2 rows 38:58
                ("f", 1, 0),    # fp8 img1 (full)
                ("x", 2, 0),    # img2 chunk0
                ("x", 2, 18),   # img2 chunk1
                ("x", 2, 38),   # img2 chunk2
                ("f", 2, 0),    # fp8 img2 (full)
                ("x", 3, 0),    # img3 chunk0
                ("x", 3, 18),   # img3 chunk1
                ("x", 3, 38),   # img3 chunk2
                ("f", 3, 0),    # fp8 img3 (full)
            ]
            # hook -1 (w g0 taps 3:9, 196K) releases EARLY (warmup #28):
            # its transfer takes ~2.3us and band 0's taps 3..8 are needed
            # within ~1us of the first real matmul; sync's queue is idle
            # by then so it pollutes nothing critical. The rest release at
            # the chain's end, when the critical head sems have fired.
            hook_pos = {28: -1}
            for j in range(len(hooks)):
                hook_pos[N_WARM - len(hooks) + j] = j
            for i in range(N_WARM):
                j = hook_pos.get(i)
                if j is None:
                    rhs = wu[:]
                    lhsT = wu[:, 0:64]
                elif j == -1:
                    rhs = w_tile[:, 0, 3, 0:NWU]
                    lhsT = wu[:, 0:64]
                else:
                    kind, a, r = hooks[j]
                    lhsT = wu[:, 0:64]
                    if kind == "x":
                        rhs = x_tiles[a][:, r, 0:NWU]
                    elif kind == "w":
                        rhs = w_tile[:, a, r, 0:NWU]
                    elif kind == "w8":
                        rhs = w8_tile[:, 0, 0, 0:NWU]
                        lhsT = wu8[:, 0:64]
                    else:  # "f": fp8 x copy
                        rhs = x8_tiles[a][:, r, 0:NWU]
                        lhsT = wu8[:, 0:64]
                nc.tensor.matmul(
                    wu_ps[:, 0 : rhs.free_size()], lhsT, rhs, start=True, stop=True
                )

            # chunk rc of image 0: band-aligned row ranges. Band rc needs
            # padded rows [rc*ROWS, rc*ROWS+ROWS+2); chunk 0 covers rows
            # 0..9, chunk rc>=1 adds rows rc*ROWS+2 .. rc*ROWS+9.
            def load_chunk(eng, b, rc):
                lo = 0 if rc == 0 else rc * ROWS + 2
                hi = rc * ROWS + ROWS + 2
                eng.dma_start(x_tiles[b][:, lo:hi], xp[b, :, lo:hi])

            # images 1..3 load in 3 tall chunks (rows 0:18 / 18:38 / 38:58):
            # a transfer's rate grows with the per-partition contiguous run
            # length, and these land ~10-30us before first use.
            IMG_CHUNKS = [(0, 18), (18, 38), (38, 58)]

            def load_img_chunk(eng, b, c):
                lo, hi = IMG_CHUNKS[c]
                eng.dma_start(x_tiles[b][:, lo:hi], xp[b, :, lo:hi])

            # Head DMAs (unhooked, start ASAP): ONLY what the first matmul
            # needs - x band-0 rows + first weight taps. Partition splits
            # MUST be 64/64: the HWDGE trigger ucode decomposes partition
            # counts into power-of-2 blocks (a 56/72-partition trigger
            # costs 1-1.8us vs ~0.63us for 64).
            def load_f8(eng, b, lo, hi):
                eng.dma_start(x8_tiles[b][:, lo:hi], xp8[b, :, lo:hi])

            nc.sync.dma_start(x_tiles[0][0:64, 0:10], xp[0, 0:64, 0:10])
            nc.scalar.dma_start(x_tiles[0][64:128, 0:10], xp[0, 64:128, 0:10])
            nc.scalar.dma_start(w_tile[:, 0, 0:3], wt[:, 0, 0:3])
            nc.sync.dma_start(b_tile[:], bz[:])
            # Hook-gated prefetches, ordered per engine by release time
            # (each engine executes its triggers in order; a gated trigger
            # blocks later ones on the same engine).
            nc.sync.dma_start(w_tile[:, 0, 3:9], wt[:, 0, 3:9])
            load_chunk(nc.scalar, 0, 1)       # j0
            load_chunk(nc.sync, 0, 2)         # j1
            load_chunk(nc.scalar, 0, 3)       # j2
            load_img_chunk(nc.sync, 1, 0)     # j3
            load_chunk(nc.scalar, 0, 4)       # j4
            load_f8(nc.sync, 0, 0, 8)         # j5
            nc.scalar.dma_start(w8_tile[:], w8[:])  # j6
            load_chunk(nc.sync, 0, 5)         # j7
            load_f8(nc.scalar, 0, 8, 24)      # j8
            load_chunk(nc.sync, 0, 6)         # j9
            load_img_chunk(nc.scalar, 1, 1)   # j10
            nc.sync.dma_start(w_tile[:, 1], wt[:, 1])  # j11
            load_f8(nc.scalar, 0, 24, 56)     # j12
            load_img_chunk(nc.sync, 1, 2)     # j13
            load_f8(nc.scalar, 1, 0, 56)      # j14
            load_img_chunk(nc.sync, 2, 0)     # j15
            load_img_chunk(nc.scalar, 2, 1)   # j16
            load_img_chunk(nc.sync, 2, 2)     # j17
            load_f8(nc.scalar, 2, 0, 56)      # j18
            load_img_chunk(nc.sync, 3, 0)     # j19
            load_img_chunk(nc.scalar, 3, 1)   # j20
            load_img_chunk(nc.sync, 3, 2)     # j21
            load_f8(nc.scalar, 3, 0, 56)      # j22

            for b in range(B_LOC):
                for g in range(NGRP):
                    for rc in range(NCHUNK):
                        ps = pspool.tile(
                            [128, NFREE], mybir.dt.float32, tag="ps", bufs=7
                        )
                        for idx, (kh, kw) in enumerate(BF16_TAPS):
                            nc.tensor.matmul(
                                ps[:],
                                w_tile[:, g, kh * KW + kw],
                                x_tiles[b][
                                    :,
                                    rc * ROWS + kh : rc * ROWS + kh + ROWS,
                                    kw : kw + W,
                                ],
                                start=(idx == 0),
                                stop=False,
                            )
                        # taps (0,1),(1,1) as one fp8 DoubleRow matmul:
                        # rhs pair = (row r, row r+1) of the pitch-56 fp8
                        # copy; 3D AP [128, (2 @56), (448 @1)] (a 4D rhs
                        # wedges the DMA^W PE - HW requires [Ki, 2, N]).
                        dr_rhs = (
                            x8_tiles[b][:, rc * ROWS : rc * ROWS + ROWS, :]
                            .unsqueeze(1)
                            .broadcast_to([128, 2, ROWS, X8P])
                        )
                        dr_rhs.ap = mybir.VecI64Pair(
                            [[H * X8P, 128], [1, 2], [NFREE, 1], [2, NFREE]]
                        )
                        dr_rhs = dr_rhs.squeeze(2)
                        nc.tensor.matmul(
                            ps[:],
                            w8_tile[:, g],
                            dr_rhs,
                            start=False,
                            stop=True,
                            perf_mode=mybir.MatmulPerfMode.DoubleRow,
                        )
                        o_tile = opool.tile(
                            [128, NFREE],
                            OUT_DT,
                            name=f"o_{b}_{g}_{rc}",
                            tag="ot",
                        )
                        # alternate eviction engine: scalar and vector can
                        # read PSUM concurrently (different banks). Vector
                        # takes even bands (it is faster: 451ns vs 635ns per
                        # 448-col eviction) INCLUDING the final band, whose
                        # eviction is on the kernel's critical tail.
                        if rc % 2 == 0:
                            nc.vector.tensor_scalar_add(
                                o_tile[:], ps[:], b_tile[:, g : g + 1]
                            )
                        else:
                            nc.scalar.activation(
                                o_tile[:],
                                ps[:],
                                mybir.ActivationFunctionType.Identity,
                                bias=b_tile[:, g : g + 1],
                                scale=1.0,
                            )
                        last_band = (
                            b == B_LOC - 1 and g == NGRP - 1 and rc == NCHUNK - 1
                        )
                        if last_band:
                            # tail: split the final out-DMA across BOTH
                            # queues BY PARTITION (keeps the 896B runs -
                            # a column split halves packet size and with
                            # it the per-queue rate, gaining nothing).
                            col0 = rc * NFREE
                            nc.sync.dma_start(
                                out[b, g, 0:64, col0 : col0 + NFREE],
                                o_tile[0:64, :],
                            )
                            nc.scalar.dma_start(
                                out[b, g, 64:128, col0 : col0 + NFREE],
                                o_tile[64:128, :],
                            )
                        else:
                            nc.sync.dma_start(
                                out[b, g, :, rc * NFREE : (rc + 1) * NFREE],
                                o_tile[:],
                            )

    _strip_mm_tick_updates(nc)
    nc.finalize()
    return nc


_NC = None


def _prep_inputs(x, weight, bias):
    x = np.asarray(x, dtype=np.float32)
    weight = np.asarray(weight, dtype=np.float32)
    bias = np.asarray(bias, dtype=np.float32)
    xp = np.zeros((B, C_IN, HP, WPAD), dtype=BF16)
    xp[:, :, 1 : H + 1, 1 : W + 1] = x.astype(BF16)
    # wt[p, g, kh*3+kw, o'] = weight[g*128+o', p, kh, kw]
    wt = np.ascontiguousarray(
        weight.reshape(NGRP, 128, C_IN, KH * KW)
        .transpose(2, 0, 3, 1)
        .astype(BF16)
    )
    # fp8 interleaved copy of x: x8i[r, c, i] = x[r+i-1, c]/8 (adjacent
    # pair bytes for DoubleRow; scaled 1/8 into e4m3's normal range, the
    # DR weights carry the inverse 8x; vertical halo baked in).
    xs = (x * 0.125).astype(F8)
    x8i = np.zeros((B, C_IN, H, W, 2), dtype=F8)
    x8i[:, :, 1:, :, 0] = xs[:, :, : H - 1]
    x8i[:, :, :, :, 1] = xs
    xp8 = x8i.reshape(B, C_IN, H, X8P)
    # w8[p, g, i, o'] = weight[g*128+o', p, kh=i, kw=1] * 8
    w8 = np.ascontiguousarray(
        (weight[:, :, 0:2, 1] * 8.0)
        .reshape(NGRP, 128, C_IN, 2)
        .transpose(2, 0, 3, 1)
        .astype(F8)
    )
    # bz[p, g] = bias[g*128 + p]
    bz = np.ascontiguousarray(bias.reshape(NGRP, 128).T)
    return xp, wt, xp8, w8, bz


def kernel(x, weight, bias, trace=False):
    global _NC
    xp, wt, xp8, w8, bz = _prep_inputs(x, weight, bias)
    if _NC is None:
        _NC = _build()
    in_maps = [
        {
            "xp": xp[c * B_LOC : (c + 1) * B_LOC],
            "wt": wt,
            "xp8": xp8[c * B_LOC : (c + 1) * B_LOC],
            "w8": w8,
            "bz": bz,
        }
        for c in range(N_CORES)
    ]
    res = run_bass_kernel_spmd(
        _NC, in_maps, core_ids=list(range(N_CORES)), trace=trace
    )
    outs = [
        r["out"].astype(np.float32).reshape(B_LOC, C_OUT, H, W) for r in res.results
    ]
    full = np.concatenate(outs, axis=0)
    if trace:
        return full, res
    return full

